# revision 1
# baseline (speedup 1.0000x reference)
"""Expert-parallel MoE routing kernel for Trainium2 (8 NeuronCores).

Problem: out[t] = x[t] @ W[idx[t]].T + b[idx[t]],  idx = pointer_addresses % 8
  x: [2048, 512] f32, W: [8, 8192, 512] f32, b: [8, 8192] f32 -> out [2048, 8192] f32

Sharding: expert-parallel. The host computes idx, gathers each expert's tokens
(padded to a common capacity `cap`), and core e computes
  out_e = x_e @ W[e].T + b[e]
with the vocab dim on PSUM partitions so the bias is a fused per-partition
bias on the Scalar/Vector engines. The host scatters rows back.

Per-core design (measured ~46 us/exec on 8 concurrent cores, vs a ~41 us
practical DMA roofline for the fp16 version of this traffic):

- W is quantized host-side to int8 (symmetric, per-expert scale
  q_e = max|W[e]|/127) and DMA'd with an int8->fp16 cast (SWDGE/gpsimd),
  HALVING the dominant HBM stream (8.4 MB -> 4.2 MB per core). The dequant
  scale is folded into the tokens: xt_e = x_e * q_e, so the NEFF is
  identical across cores (SPMD) and no on-device rescale is needed.
  Quantization error ~3.4e-3 max-rel (threshold 2e-2), since uniform W
  quantizes ~9x more accurately in int8 than fp8.
- Matmul orientation (out = lhsT.T @ rhs): lhsT = W chunk [K=128, M=128]
  stationary, rhs = xT chunk [K=128, N=cap] moving, PSUM [128 vocab, cap]
  accumulated over 4 K-chunks of D=512.
- W loads are grouped [4,12,16,16,16] vocab-chunks (small first so the PE
  stream starts ~2 us in, large after so the DMA ring stays efficient);
  out stores are grouped [16,16,16,12,4] (small last so the post-compute
  store tail is short). Groups are decoupled.
- Engine layout: W loads on gpsimd (SWDGE, cast), out stores on sync
  (SP HWDGE, otherwise idle), PSUM evictions alternate Scalar/Vector, with
  the tail group forced to Vector so the last store is not queued behind
  ACT work.
"""

import contextlib

import numpy as np

E = 8          # experts == cores
D = 512        # hidden
V = 8192       # out features
P = 128        # partitions
KCH = D // P   # 4 contraction chunks
VCH = V // P   # 64 vocab chunks

SCHED_W = (4, 12, 16, 16, 16)   # W-load groups (vocab chunks)
SCHED_O = (16, 16, 16, 12, 4)   # out-store groups
WP_BUFS = 6

LAST_RESULT = None  # BassKernelResults of the most recent run (for harness)

_BUILD_CACHE = {}


def _build(cap, loop_n=1):
    """Build the per-core Bass module for token capacity `cap`.

    loop_n > 1 wraps the compute in a hardware For_i re-running it (same
    outputs) so a test harness can difference wall-times to isolate the
    on-device per-execution time.
    """
    key = (cap, loop_n)
    if key in _BUILD_CACHE:
        return _BUILD_CACHE[key]

    import concourse.mybir as mybir
    from concourse import bacc
    from concourse.tile import TileContext

    i8 = mybir.dt.int8
    fp16 = mybir.dt.float16
    f32 = mybir.dt.float32
    gw = max(SCHED_W)
    go = max(SCHED_O)

    nc = bacc.Bacc(None, target_bir_lowering=False)
    # flat layouts, vocab-chunk (vi) as the per-partition-contiguous axis
    wt = nc.dram_tensor("wt", [P, VCH, KCH, P], i8, kind="ExternalInput")
    xt = nc.dram_tensor("xt", [P, KCH, cap], fp16, kind="ExternalInput")
    bias = nc.dram_tensor("bias", [P, VCH], f32, kind="ExternalInput")
    out = nc.dram_tensor("out", [P, VCH, cap], fp16, kind="ExternalOutput")

    # token chunks of <=512 (PSUM bank f32 limit)
    t_chunks = []
    t0 = 0
    while t0 < cap:
        t_chunks.append((t0, min(512, cap - t0)))
        t0 += 512

    w_start, o_start, o_end = {}, {}, {}
    s = 0
    for n in SCHED_W:
        w_start[s] = n
        s += n
    s = 0
    for n in SCHED_O:
        o_start[s] = n
        o_end[s + n] = (s, n)
        s += n

    def body():
        w_sb = o_sb = None
        w0 = o0 = 0
        for vi in range(VCH):
            if vi in w_start:
                nvi = w_start[vi]
                w_full = wp.tile([P, gw, KCH, P], fp16, tag="w", name="w_full")
                w_sb = w_full[:, :nvi]
                # int8 DRAM -> fp16 SBUF cast during the SWDGE DMA
                nc.gpsimd.dma_start(w_sb, wt.ap()[:, vi : vi + nvi])
                w0 = vi
            if vi in o_start:
                o_full = op_.tile([P, go, cap], fp16, tag="o", name="o_full")
                o_sb = o_full[:, : o_start[vi]]
                o0 = vi
            for tc0, tw in t_chunks:
                ps = pp.tile([P, tw], f32, tag="ps")
                for k in range(KCH):
                    nc.tensor.matmul(
                        ps,
                        lhsT=w_sb[:, vi - w0, k],
                        rhs=x_sb[:, k, tc0 : tc0 + tw],
                        start=(k == 0),
                        stop=(k == KCH - 1),
                    )
                # out = psum + bias; alternate ScalarE / VectorE so neither
                # engine's eviction throughput becomes the bottleneck; the
                # tail store-group goes all-Vector so the final SP store
                # isn't serialized behind ACT evictions
                if vi >= VCH - SCHED_O[-1] or vi % 2:
                    nc.vector.tensor_tensor(
                        o_sb[:, vi - o0, tc0 : tc0 + tw], ps,
                        b_sb[:, vi : vi + 1].to_broadcast((P, tw)),
                        mybir.AluOpType.add,
                    )
                else:
                    nc.scalar.activation(
                        o_sb[:, vi - o0, tc0 : tc0 + tw], ps,
                        mybir.ActivationFunctionType.Identity,
                        bias=b_sb[:, vi : vi + 1], scale=1.0,
                    )
            if vi + 1 in o_end:
                b0, n = o_end[vi + 1]
                nc.sync.dma_start(out.ap()[:, b0 : b0 + n], o_sb)

    with TileContext(nc) as tc:
        with (
            tc.tile_pool(name="xp", bufs=1) as xp,
            tc.tile_pool(name="bp", bufs=1) as bp,
            tc.tile_pool(name="wp", bufs=WP_BUFS) as wp,
            tc.tile_pool(name="op", bufs=3) as op_,
            tc.tile_pool(name="pp", bufs=8, space="PSUM") as pp,
        ):
            x_sb = xp.tile([P, KCH, cap], fp16)
            nc.gpsimd.dma_start(x_sb, xt.ap())
            b_sb = bp.tile([P, VCH], f32)
            nc.gpsimd.dma_start(b_sb, bias.ap())

            loop_cm = (
                tc.For_i(0, loop_n, 1) if loop_n > 1 else contextlib.nullcontext()
            )
            with loop_cm:
                body()

    nc.finalize()
    _BUILD_CACHE[key] = nc
    return nc


def _prepare(x, pointer_addresses, W, b):
    """Host-side shard: gather tokens per expert, quantize W, scale tokens."""
    x = np.ascontiguousarray(np.asarray(x), dtype=np.float32)
    W = np.ascontiguousarray(np.asarray(W), dtype=np.float32)
    b = np.ascontiguousarray(np.asarray(b), dtype=np.float32)
    pa = np.asarray(pointer_addresses)

    idx = (pa.astype(np.int64) % E).astype(np.int64)
    rows = [np.flatnonzero(idx == e) for e in range(E)]
    counts = np.array([len(r) for r in rows])
    cap = max(256, int(counts.max()))

    in_maps = []
    for e in range(E):
        q = float(np.abs(W[e]).max()) / 127.0
        if q == 0.0:
            q = 1.0
        wq = np.clip(np.round(W[e] / q), -127, 127).astype(np.int8)
        # wt: [p, vi, k, c] = Wq[vi*P + c, k*P + p]
        w_e = np.ascontiguousarray(
            wq.reshape(VCH, P, KCH, P).transpose(3, 0, 2, 1)
        )
        # xT: [P(d inner), KCH, cap], pre-scaled by the dequant factor q
        x_pad = np.zeros((cap, D), np.float32)
        x_pad[: counts[e]] = x[rows[e]] * q
        xt_e = np.ascontiguousarray(
            x_pad.reshape(cap, KCH, P).transpose(2, 1, 0).astype(np.float16)
        )
        # bias: [P(c), VCH]
        b_e = np.ascontiguousarray(b[e].reshape(VCH, P).T)
        in_maps.append({"wt": w_e, "xt": xt_e, "bias": b_e})

    return in_maps, rows, counts, cap


def _run(nc, in_maps):
    global LAST_RESULT
    from concourse.bass_utils import run_bass_kernel_spmd

    res = run_bass_kernel_spmd(nc, in_maps, core_ids=list(range(E)))
    LAST_RESULT = res
    return res


def _assemble(res, rows, counts, cap, n_tokens):
    out = np.zeros((n_tokens, V), np.float32)
    for e in range(E):
        # out dram [P(c), VCH, cap] -> vocab-major [V, cap]
        o = (
            res.results[e]["out"]
            .astype(np.float32)
            .transpose(1, 0, 2)
            .reshape(V, cap)
        )
        out[rows[e]] = o[:, : counts[e]].T
    return out


def kernel(x, pointer_addresses, W, b):
    in_maps, rows, counts, cap = _prepare(x, pointer_addresses, W, b)
    nc = _build(cap)
    res = _run(nc, in_maps)
    return _assemble(res, rows, counts, cap, np.asarray(x).shape[0])



# revision 2
# speedup vs baseline: 7.1511x; 7.1511x over previous
"""MoE routing kernel for Trainium2 (8 NeuronCores), v3: vocab-sharded
all-expert layout ("OCT split").

Problem: out[t] = x[t] @ W[idx[t]].T + b[idx[t]],  idx = pointer_addresses % 8
  x: [2048, 512] f32, W: [8, 8192, 512] f32, b: [8, 8192] f32 -> out [2048, 8192] f32

Sharding: every core holds a V/8 = 1024-row vocab slice of ALL 8 experts
and processes ALL tokens (grouped by expert, exact counts, no padding).
Because every core sees the same token partition, the per-core instruction
stream (and all access-pattern extents) is identical across cores -> SPMD,
while the PE work drops to sum_e 32chunks*c_e = 65,536 cycles (27.3 us),
perfectly balanced regardless of routing skew.

Numerics: W is quantized to fp8 e3m4 per (expert, core-slice) with scale
q = max|W_slice|/15.5; x is quantized to fp8 e3m4 at unit scale; the PE
accumulates fp32; the eviction adds a pre-divided bias b' = b/q and stores
fp16; the host multiplies each (expert, slice) output block by q during
assembly. End-to-end max-rel error vs the fp32 reference: 1.58e-2 (< 2e-2),
measured exactly on the fixed-seed inputs.

Per-core DMA: W 4.19 MB + x 1.05 MB + out 4.19 MB -> ~26.3 us of DMA device
busy vs 27.3 us PE busy. A PE-pstate warmup chain of dummy matmuls covers
the cost model's 3 us ramp during the DMA head.
"""

import contextlib

import numpy as np

E = 8          # experts
D = 512        # hidden
V = 8192       # out features
P = 128        # partitions
KCH = D // P   # 4 contraction chunks
VCH = 64       # vocab chunks per core (V/8 rows = 8 chunks per expert * 8 experts)
VS = V // E    # vocab slice per core (1024)
T = 2048       # total tokens

F8_MAX = 15.5  # largest finite float8_e3m4

# W-load groups (in vocab chunks), all issued from gpsimd in order; the
# DMA device serves them interleaved with the small SP-issued x segments.
SCHED_W = (6, 9, 12, 18, 19)
# out-store groups (in vocab chunks)
SCHED_O = (12, 12, 12, 12, 10, 4, 2)
# x segments by expert ranges, all issued upfront on sync (SP)
SCHED_X = tuple((e, e + 1) for e in range(E))
WP_BUFS = 6
OP_BUFS = 4
N_WARM = 14
WARM_COLS = 256

LAST_RESULT = None

_BUILD_CACHE = {}


def _build(counts, loop_n=1):
    """Build the per-core Bass module for the (shared) per-expert counts."""
    counts = tuple(int(c) for c in counts)
    key = (counts, loop_n)
    if key in _BUILD_CACHE:
        return _BUILD_CACHE[key]

    import concourse.mybir as mybir
    from concourse import bacc
    from concourse.tile import TileContext

    f8 = mybir.dt.float8e3
    fp16 = mybir.dt.float16
    f32 = mybir.dt.float32

    t_total = sum(counts)
    offx = np.concatenate([[0], np.cumsum(counts)])          # token offsets
    c_vi = [counts[vi // 8] for vi in range(VCH)]
    offo = np.concatenate([[0], np.cumsum(c_vi)])            # out flat offsets
    n_out = int(offo[-1])
    gw = max(SCHED_W)

    nc = bacc.Bacc(None, target_bir_lowering=False)
    wt = nc.dram_tensor("wt", [P, VCH, KCH, P], f8, kind="ExternalInput")
    xt = nc.dram_tensor("xt", [P, t_total, KCH], f8, kind="ExternalInput")
    bias = nc.dram_tensor("bias", [P, VCH], f32, kind="ExternalInput")
    out = nc.dram_tensor("out", [P, n_out], fp16, kind="ExternalOutput")

    w_start, o_start, o_end = {}, {}, {}
    s = 0
    for n in SCHED_W:
        w_start[s] = n
        s += n
    s = 0
    for n in SCHED_O:
        o_start[s] = n
        o_end[s + n] = (s, n)
        s += n

    # x segment lookup: expert -> (tile index, local col offset)
    xseg_of_expert = {}
    for si, (e0, e1) in enumerate(SCHED_X):
        for e in range(e0, e1):
            xseg_of_expert[e] = (si, int(offx[e] - offx[e0]))

    def body():
        w_sb = o_sb = None
        w0 = o0 = 0
        wgi = 0
        for vi in range(VCH):
            e = vi // 8
            cw = c_vi[vi]
            if vi in w_start:
                nvi = w_start[vi]
                w_full = wp.tile([P, gw, KCH, P], f8, tag="w", name="w_full")
                w_sb = w_full[:, :nvi]
                # all W groups on gpsimd: its issue chain runs parallel to
                # the SP chain (bias + x segments), and the DMA device picks
                # them up interleaved by arrival time
                nc.gpsimd.dma_start(w_sb, wt.ap()[:, vi : vi + nvi])
                w0 = vi
                wgi += 1
            if vi in o_start:
                L = int(offo[vi + o_start[vi]] - offo[vi])
                o_full = op_.tile([P, L], fp16, tag="o", name="o_full")
                o_sb = o_full
                o0 = vi
            si, tloc = xseg_of_expert[e]
            x_sb = x_tiles[si]
            ps = pp.tile([P, cw], f32, tag="ps")
            for k in range(KCH):
                nc.tensor.matmul(
                    ps,
                    lhsT=w_sb[:, vi - w0, k],
                    rhs=x_sb[:, tloc : tloc + cw, k],
                    start=(k == 0),
                    stop=(k == KCH - 1),
                )
            oc0 = int(offo[vi] - offo[o0])
            if vi >= VCH - (SCHED_O[-1] + SCHED_O[-2]) or vi % 2:
                nc.vector.tensor_tensor(
                    o_sb[:, oc0 : oc0 + cw], ps,
                    b_sb[:, vi : vi + 1].to_broadcast((P, cw)),
                    mybir.AluOpType.add,
                )
            else:
                nc.scalar.activation(
                    o_sb[:, oc0 : oc0 + cw], ps,
                    mybir.ActivationFunctionType.Identity,
                    bias=b_sb[:, vi : vi + 1], scale=1.0,
                )
            if vi + 1 in o_end:
                b0, n = o_end[vi + 1]
                nc.sync.dma_start(
                    out.ap()[:, int(offo[b0]) : int(offo[b0 + n])], o_sb
                )

    with TileContext(nc) as tc:
        with (
            tc.tile_pool(name="xp", bufs=len(SCHED_X)) as xp,
            tc.tile_pool(name="warm", bufs=1) as warmp,
            tc.tile_pool(name="bp", bufs=1) as bp,
            tc.tile_pool(name="wp", bufs=WP_BUFS) as wp,
            tc.tile_pool(name="op", bufs=OP_BUFS) as op_,
            tc.tile_pool(name="pp", bufs=8, space="PSUM") as pp,
        ):
            # bias first on SP (tiny), then the x segment tiles in expert
            # order (fp8, [P, seg_tokens, KCH]) — all upfront, before any
            # store lands on the SP queue.
            b_sb = bp.tile([P, VCH], f32)
            nc.sync.dma_start(b_sb, bias.ap())
            x_tiles = []
            for si, (e0, e1) in enumerate(SCHED_X):
                seg = int(offx[e1] - offx[e0])
                xtile = xp.tile([P, seg, KCH], f8, name=f"xseg{si}")
                x_tiles.append(xtile)
                nc.sync.dma_start(xtile, xt.ap()[:, int(offx[e0]) : int(offx[e1])])

            # PE pstate warm-up (see kernel_v2)
            if N_WARM:
                wm = warmp.tile([P, WARM_COLS], fp16, name="warm")
                nc.vector.memset(wm, 0)
                for _ in range(N_WARM):
                    psw = pp.tile([P, WARM_COLS], f32, tag="ps", name="ps_warm")
                    nc.tensor.matmul(
                        psw, lhsT=wm[:, :P], rhs=wm, start=True, stop=True
                    )

            loop_cm = (
                tc.For_i(0, loop_n, 1) if loop_n > 1 else contextlib.nullcontext()
            )
            with loop_cm:
                body()

    nc.finalize()
    _BUILD_CACHE[key] = nc
    return nc


def _prepare(x, pointer_addresses, W, b):
    """Host-side prep: token grouping, fp8 quantization, per-core layouts."""
    import ml_dtypes

    f8 = ml_dtypes.float8_e3m4
    x = np.ascontiguousarray(np.asarray(x), dtype=np.float32)
    W = np.ascontiguousarray(np.asarray(W), dtype=np.float32)
    b = np.ascontiguousarray(np.asarray(b), dtype=np.float32)
    pa = np.asarray(pointer_addresses)

    idx = (pa.astype(np.int64) % E).astype(np.int64)
    rows = [np.flatnonzero(idx == e) for e in range(E)]
    counts = tuple(int(len(r)) for r in rows)
    t_total = sum(counts)

    order = np.concatenate(rows) if t_total else np.zeros(0, np.int64)
    xq = x[order].astype(f8)                     # [T, D]
    # xt[p, t, k] = xq[t, k*128+p]
    xt = np.ascontiguousarray(xq.reshape(t_total, KCH, P).transpose(2, 0, 1))

    in_maps = []
    scales = np.zeros((E, E), np.float32)        # [e, core j]
    for j in range(E):
        wt_j = np.zeros((P, VCH, KCH, P), f8)
        bias_j = np.zeros((P, VCH), np.float32)
        for e in range(E):
            sl = W[e][j * VS : (j + 1) * VS]     # [1024, 512]
            q = float(np.abs(sl).max()) / F8_MAX
            if q == 0.0:
                q = 1.0
            scales[e, j] = q
            wq = (sl / q).astype(f8)
            # [s, c, k, p] -> [p, s, k, c]
            a = wq.reshape(8, P, KCH, P).transpose(3, 0, 2, 1)
            wt_j[:, e * 8 : (e + 1) * 8] = a
            bias_j[:, e * 8 : (e + 1) * 8] = (
                (b[e][j * VS : (j + 1) * VS] / q).reshape(8, P).T
            )
        in_maps.append({"wt": wt_j, "xt": xt, "bias": bias_j})

    return in_maps, rows, counts, scales


def _run(nc, in_maps):
    global LAST_RESULT
    from concourse.bass_utils import run_bass_kernel_spmd

    res = run_bass_kernel_spmd(nc, in_maps, core_ids=list(range(E)))
    LAST_RESULT = res
    return res


def _assemble(res, rows, counts, scales, n_tokens):
    c_vi = [counts[vi // 8] for vi in range(VCH)]
    offo = np.concatenate([[0], np.cumsum(c_vi)])
    out = np.zeros((n_tokens, V), np.float32)
    for j in range(E):
        o = res.results[j]["out"].astype(np.float32)   # [P, n_out]
        for e in range(E):
            if counts[e] == 0:
                continue
            blk = np.empty((counts[e], VS), np.float32)
            for s in range(8):
                vi = e * 8 + s
                seg = o[:, int(offo[vi]) : int(offo[vi + 1])]  # [P, c_e]
                blk[:, s * P : (s + 1) * P] = seg.T
            out[np.ix_(rows[e], np.arange(j * VS, (j + 1) * VS))] = (
                blk * scales[e, j]
            )
    return out


def kernel(x, pointer_addresses, W, b):
    in_maps, rows, counts, scales = _prepare(x, pointer_addresses, W, b)
    nc = _build(counts)
    res = _run(nc, in_maps)
    return _assemble(res, rows, counts, scales, np.asarray(x).shape[0])


# revision 4
# speedup vs baseline: 7.3059x; 1.0216x over previous
"""MoE routing kernel for Trainium2 (8 NeuronCores), v3: vocab-sharded
all-expert layout ("OCT split").

Problem: out[t] = x[t] @ W[idx[t]].T + b[idx[t]],  idx = pointer_addresses % 8
  x: [2048, 512] f32, W: [8, 8192, 512] f32, b: [8, 8192] f32 -> out [2048, 8192] f32

Sharding: every core holds a V/8 = 1024-row vocab slice of ALL 8 experts
and processes ALL tokens (grouped by expert, exact counts, no padding).
Because every core sees the same token partition, the per-core instruction
stream (and all access-pattern extents) is identical across cores -> SPMD,
while the PE work drops to sum_e 32chunks*c_e = 65,536 cycles (27.3 us),
perfectly balanced regardless of routing skew.

Numerics: W is quantized to fp8 e3m4 per (expert, core-slice) with scale
q = max|W_slice|/15.5; x is quantized to fp8 e3m4 at unit scale; the PE
accumulates fp32; the eviction adds a pre-divided bias b' = b/q and stores
fp16; the host multiplies each (expert, slice) output block by q during
assembly. End-to-end max-rel error vs the fp32 reference: 1.58e-2 (< 2e-2),
measured exactly on the fixed-seed inputs.

Per-core DMA: W 4.19 MB + x 1.05 MB + out 4.19 MB -> ~26.3 us of DMA device
busy vs 27.3 us PE busy. A PE-pstate warmup chain of dummy matmuls covers
the cost model's 3 us ramp during the DMA head.
"""

import contextlib

import numpy as np

E = 8          # experts
D = 512        # hidden
V = 8192       # out features
P = 128        # partitions
KCH = D // P   # 4 contraction chunks
VCH = 64       # vocab chunks per core (V/8 rows = 8 chunks per expert * 8 experts)
VS = V // E    # vocab slice per core (1024)
T = 2048       # total tokens

F8_MAX = 15.5  # largest finite float8_e3m4

# W-load groups (in vocab chunks) and the engine issuing each: the first
# groups fan out across pool/scalar/vector issue chains so their DMAs reach
# the (serialized) DMA device back-to-back at the start; later groups stay
# on gpsimd. The DMA device serves everything interleaved by arrival time.
SCHED_W = (2, 3, 5, 9, 12, 16, 17)
SCHED_W_ENG = ("pool", "act", "pool", "pool", "pool", "pool", "pool")
# split the last vocab chunk's eviction across DVE+ACT (parallel halves);
# measured slower in the cost model (extra sem prop on the store's waits),
# kept as an option but off
SPLIT_LAST_EVICT = False
# out-store groups (in vocab chunks)
SCHED_O = (12, 12, 12, 12, 10, 4, 2)
# x segments by expert ranges, all issued upfront on sync (SP)
SCHED_X = tuple((e, e + 1) for e in range(E))
WP_BUFS = 6
OP_BUFS = 4
N_WARM = 14
WARM_COLS = 192

LAST_RESULT = None

_BUILD_CACHE = {}


def _build(counts, loop_n=1):
    """Build the per-core Bass module for the (shared) per-expert counts."""
    counts = tuple(int(c) for c in counts)
    key = (counts, loop_n)
    if key in _BUILD_CACHE:
        return _BUILD_CACHE[key]

    import concourse.mybir as mybir
    from concourse import bacc
    from concourse.tile import TileContext

    f8 = mybir.dt.float8e3
    fp16 = mybir.dt.float16
    f32 = mybir.dt.float32

    t_total = sum(counts)
    offx = np.concatenate([[0], np.cumsum(counts)])          # token offsets
    c_vi = [counts[vi // 8] for vi in range(VCH)]
    offo = np.concatenate([[0], np.cumsum(c_vi)])            # out flat offsets
    n_out = int(offo[-1])
    gw = max(SCHED_W)

    nc = bacc.Bacc(None, target_bir_lowering=False)
    wt = nc.dram_tensor("wt", [P, VCH, KCH, P], f8, kind="ExternalInput")
    xt = nc.dram_tensor("xt", [P, t_total, KCH], f8, kind="ExternalInput")
    bias = nc.dram_tensor("bias", [P, VCH], f32, kind="ExternalInput")
    out = nc.dram_tensor("out", [P, n_out], fp16, kind="ExternalOutput")

    w_start, o_start, o_end = {}, {}, {}
    s = 0
    for n in SCHED_W:
        w_start[s] = n
        s += n
    s = 0
    for n in SCHED_O:
        o_start[s] = n
        o_end[s + n] = (s, n)
        s += n

    # x segment lookup: expert -> (tile index, local col offset)
    xseg_of_expert = {}
    for si, (e0, e1) in enumerate(SCHED_X):
        for e in range(e0, e1):
            xseg_of_expert[e] = (si, int(offx[e] - offx[e0]))

    def body():
        w_sb = o_sb = None
        w0 = o0 = 0
        wgi = 0
        for vi in range(VCH):
            e = vi // 8
            cw = c_vi[vi]
            if vi in w_start:
                nvi = w_start[vi]
                if wgi in pre_w:
                    # group pre-issued from the preamble (non-pool engine:
                    # its DMA had to be emitted before any eviction landed
                    # on that engine's queue)
                    w_sb = pre_w[wgi]
                else:
                    w_full = wp.tile([P, gw, KCH, P], f8, tag="w", name="w_full")
                    w_sb = w_full[:, :nvi]
                    nc.gpsimd.dma_start(w_sb, wt.ap()[:, vi : vi + nvi])
                w0 = vi
                wgi += 1
            if vi in o_start:
                L = int(offo[vi + o_start[vi]] - offo[vi])
                o_full = op_.tile([P, L], fp16, tag="o", name="o_full")
                o_sb = o_full
                o0 = vi
            si, tloc = xseg_of_expert[e]
            x_sb = x_tiles[si]
            ps = pp.tile([P, cw], f32, tag="ps")
            for k in range(KCH):
                nc.tensor.matmul(
                    ps,
                    lhsT=w_sb[:, vi - w0, k],
                    rhs=x_sb[:, tloc : tloc + cw, k],
                    start=(k == 0),
                    stop=(k == KCH - 1),
                )
            oc0 = int(offo[vi] - offo[o0])
            if SPLIT_LAST_EVICT and vi == VCH - 1:
                # final eviction sits on the store critical path: halves run
                # in parallel on DVE and ACT
                h = cw // 2
                nc.vector.tensor_tensor(
                    o_sb[:, oc0 : oc0 + h], ps[:, :h],
                    b_sb[:, vi : vi + 1].to_broadcast((P, h)),
                    mybir.AluOpType.add,
                )
                nc.scalar.activation(
                    o_sb[:, oc0 + h : oc0 + cw], ps[:, h:cw],
                    mybir.ActivationFunctionType.Identity,
                    bias=b_sb[:, vi : vi + 1], scale=1.0,
                )
            elif vi >= VCH - (SCHED_O[-1] + SCHED_O[-2]) or vi % 2:
                nc.vector.tensor_tensor(
                    o_sb[:, oc0 : oc0 + cw], ps,
                    b_sb[:, vi : vi + 1].to_broadcast((P, cw)),
                    mybir.AluOpType.add,
                )
            else:
                nc.scalar.activation(
                    o_sb[:, oc0 : oc0 + cw], ps,
                    mybir.ActivationFunctionType.Identity,
                    bias=b_sb[:, vi : vi + 1], scale=1.0,
                )
            if vi + 1 in o_end:
                b0, n = o_end[vi + 1]
                nc.sync.dma_start(
                    out.ap()[:, int(offo[b0]) : int(offo[b0 + n])], o_sb
                )

    with TileContext(nc) as tc:
        with (
            tc.tile_pool(name="xp", bufs=len(SCHED_X)) as xp,
            tc.tile_pool(name="warm", bufs=1) as warmp,
            tc.tile_pool(name="bp", bufs=1) as bp,
            tc.tile_pool(name="wp", bufs=WP_BUFS) as wp,
            tc.tile_pool(name="op", bufs=OP_BUFS) as op_,
            tc.tile_pool(name="pp", bufs=8, space="PSUM") as pp,
        ):
            # x segment 0 first on SP (the PE-start critical path), then the
            # bias (needed by the first eviction), then any SP-issued W
            # groups, then the remaining x segments — all upfront, before
            # any store lands on the SP queue. Non-pool W groups must be
            # pre-issued here: emitted later they would queue behind
            # sem-waiting evictions on their engine's FIFO sequencer.
            x_tiles = []
            b_sb = bp.tile([P, VCH], f32)
            pre_w = {}

            def pre_issue_w(which):
                s = 0
                for gi, n in enumerate(SCHED_W):
                    if SCHED_W_ENG[gi] == which:
                        w_full = wp.tile([P, gw, KCH, P], f8, tag="w", name="w_full")
                        pre_w[gi] = w_full[:, :n]
                        eng = {"act": nc.scalar, "sp": nc.sync}[which]
                        eng.dma_start(pre_w[gi], wt.ap()[:, s : s + n])
                    s += n

            for si, (e0, e1) in enumerate(SCHED_X):
                seg = int(offx[e1] - offx[e0])
                xtile = xp.tile([P, seg, KCH], f8, name=f"xseg{si}")
                x_tiles.append(xtile)
                nc.sync.dma_start(xtile, xt.ap()[:, int(offx[e0]) : int(offx[e1])])
                if si == 0:
                    nc.sync.dma_start(b_sb, bias.ap())
                    pre_issue_w("sp")
                    pre_issue_w("act")

            # PE pstate warm-up (see kernel_v2)
            if N_WARM:
                wm = warmp.tile([P, WARM_COLS], fp16, name="warm")
                nc.vector.memset(wm, 0)
                for _ in range(N_WARM):
                    psw = pp.tile([P, WARM_COLS], f32, tag="ps", name="ps_warm")
                    nc.tensor.matmul(
                        psw, lhsT=wm[:, :P], rhs=wm, start=True, stop=True
                    )

            loop_cm = (
                tc.For_i(0, loop_n, 1) if loop_n > 1 else contextlib.nullcontext()
            )
            with loop_cm:
                body()

    nc.finalize()
    _BUILD_CACHE[key] = nc
    return nc


def _prepare(x, pointer_addresses, W, b):
    """Host-side prep: token grouping, fp8 quantization, per-core layouts."""
    import ml_dtypes

    f8 = ml_dtypes.float8_e3m4
    x = np.ascontiguousarray(np.asarray(x), dtype=np.float32)
    W = np.ascontiguousarray(np.asarray(W), dtype=np.float32)
    b = np.ascontiguousarray(np.asarray(b), dtype=np.float32)
    pa = np.asarray(pointer_addresses)

    idx = (pa.astype(np.int64) % E).astype(np.int64)
    rows = [np.flatnonzero(idx == e) for e in range(E)]
    counts = tuple(int(len(r)) for r in rows)
    t_total = sum(counts)

    order = np.concatenate(rows) if t_total else np.zeros(0, np.int64)
    xq = x[order].astype(f8)                     # [T, D]
    # xt[p, t, k] = xq[t, k*128+p]
    xt = np.ascontiguousarray(xq.reshape(t_total, KCH, P).transpose(2, 0, 1))

    in_maps = []
    scales = np.zeros((E, E), np.float32)        # [e, core j]
    for j in range(E):
        wt_j = np.zeros((P, VCH, KCH, P), f8)
        bias_j = np.zeros((P, VCH), np.float32)
        for e in range(E):
            sl = W[e][j * VS : (j + 1) * VS]     # [1024, 512]
            q = float(np.abs(sl).max()) / F8_MAX
            if q == 0.0:
                q = 1.0
            scales[e, j] = q
            wq = (sl / q).astype(f8)
            # [s, c, k, p] -> [p, s, k, c]
            a = wq.reshape(8, P, KCH, P).transpose(3, 0, 2, 1)
            wt_j[:, e * 8 : (e + 1) * 8] = a
            bias_j[:, e * 8 : (e + 1) * 8] = (
                (b[e][j * VS : (j + 1) * VS] / q).reshape(8, P).T
            )
        in_maps.append({"wt": wt_j, "xt": xt, "bias": bias_j})

    return in_maps, rows, counts, scales


def _run(nc, in_maps):
    global LAST_RESULT
    from concourse.bass_utils import run_bass_kernel_spmd

    res = run_bass_kernel_spmd(nc, in_maps, core_ids=list(range(E)))
    LAST_RESULT = res
    return res


def _assemble(res, rows, counts, scales, n_tokens):
    c_vi = [counts[vi // 8] for vi in range(VCH)]
    offo = np.concatenate([[0], np.cumsum(c_vi)])
    out = np.zeros((n_tokens, V), np.float32)
    for j in range(E):
        o = res.results[j]["out"].astype(np.float32)   # [P, n_out]
        for e in range(E):
            if counts[e] == 0:
                continue
            blk = np.empty((counts[e], VS), np.float32)
            for s in range(8):
                vi = e * 8 + s
                seg = o[:, int(offo[vi]) : int(offo[vi + 1])]  # [P, c_e]
                blk[:, s * P : (s + 1) * P] = seg.T
            out[np.ix_(rows[e], np.arange(j * VS, (j + 1) * VS))] = (
                blk * scales[e, j]
            )
    return out


def kernel(x, pointer_addresses, W, b):
    in_maps, rows, counts, scales = _prepare(x, pointer_addresses, W, b)
    nc = _build(counts)
    res = _run(nc, in_maps)
    return _assemble(res, rows, counts, scales, np.asarray(x).shape[0])
